# revision 8
# baseline (speedup 1.0000x reference)
"""Trainium2 Bass kernel for BinaryLinearUnit:
    y = sign(x) @ sign(w).T ; BatchNorm1d(train) ; * gamma + beta

Strategy: 2D sharding over 8 NeuronCores — 4 batch shards x 2
output-feature shards. Each core computes y.T for its [2048 batch x
2048 out-features] block with an FP8 (DoubleRow) matmul. Versus pure
data-parallel this cuts per-core HBM traffic (the dominant baseline
cost) from ~100MB to ~32MB:
  - x ships as fp8e5m2 (sign-preserving for N(0,1) values up to a
    ~6e-6 fraction that rounds to 0; adds ~2.5e-3 rel err, well under
    tolerance), K-major: 8MB/core.
  - w ships as bf16 (bf16 cast preserves sign exactly), K-major
    packed: 16MB/core.
  - y_hat ships back as fp16 (~5e-4 rel err): 8MB/core.

Signs: x-sign on DVE via one tensor_scalar (is_ge 0, sub 0.5) giving
{-0.5,+0.5} fp8 — BatchNorm cancels any constant scale of y exactly,
so +-0.5 works as well as +-1 and needs a single instruction. w-sign
on ACT (scalar.sign) giving +-1 fp8. PSUM accumulation is fp32-exact
(y/2 is a sum of +-0.5 with |y|<=4096, and even y is exact in fp16).

BN batch stats need cross-core reduction only within each group of 4
cores that shares the same output-feature shard: partial [mean, E[y^2]]
per channel are AllGathered over replica groups [[0-3],[4-7]] and
summed locally. The output tiles are processed in NSPLIT stat groups;
each group's collective is issued as soon as its matmuls finish and its
post-collective math + normalization are anchored a few tiles later,
so all BN work except the last group's overlaps the remaining matmuls.

Engine assignment: PE matmuls | ACT w-sign + sqrt + half the output
stores | DVE x-sign, bn_stats, psum->f16 copy, stats math, normalize,
x-input DMA queue | GpSimd collectives + readback | Sync w/gb DMA +
half the output stores.
"""

import numpy as np
import ml_dtypes

import concourse.bass as bass
import concourse.mybir as mybir
import concourse.tile as tile
from concourse import bacc
from concourse.bass import ts
from concourse.bass_utils import run_bass_kernel_spmd
from concourse.tile_rust import add_dep_helper

N_CORES = 8
KB_SHARD = 4            # batch shards
KO_SHARD = 2            # output-feature shards
BN_EPS = 1e-5

f32 = mybir.dt.float32
f16 = mybir.dt.float16
bf16 = mybir.dt.bfloat16
fp8 = mybir.dt.float8e4
fp8e5 = mybir.dt.float8e5


def build(B, IN, OUT, kb=KB_SHARD, ko=KO_SHARD):
    """Per-core SPMD module. Core c handles batch shard c%kb and
    out-feature shard c//kb. Shapes: x [B, IN], w [OUT, IN]."""
    Bc = B // kb            # batch rows per core
    OUTc = OUT // ko        # out features per core
    KT = IN // 128          # k tiles (contraction)
    KP = KT // 2            # fp8 DoubleRow consumes k-pairs
    OT = OUTc // 128        # output-feature tiles per core
    NB = 512                # matmul free dim / psum bank width
    BT = Bc // NB           # b tiles per core
    n_group = N_CORES // ko  # cores sharing one out-feature shard
    groups = [[g * n_group + i for i in range(n_group)] for g in range(ko)]

    # BN stat groups over the output tiles: earlier groups' collectives
    # overlap remaining matmuls. Collectives serialize on the single CC
    # stream and cost ~21us when they absorb inter-core skew, so the
    # second-to-last group ends 2 tiles (~27us) before the matmuls do —
    # hiding its latency — and the last group is small.
    GS = [6, 6, 2, 2] if OT == 16 else [OT - OT // 2, OT // 2]
    NSPLIT = len(GS)
    GO = [sum(GS[:q]) for q in range(NSPLIT)]

    nc = bacc.Bacc("TRN2", target_bir_lowering=False, debug=False,
                   num_devices=N_CORES)

    # Per-core external I/O (host pre-transposed, K-major):
    #   xt[k, b] = x[(c%kb)*Bc + b, k]          as fp8e5m2
    #   w2[ot, p, ks, o] = w[(c//kb)*OUTc + ot*128 + o, ks*128 + p]  bf16
    #   yt[o, b] = out[(c%kb)*Bc + b, (c//kb)*OUTc + o]  fp16
    xt = nc.dram_tensor("xt", [IN, Bc], fp8e5, kind="ExternalInput")
    w2 = nc.dram_tensor("w2", [OT, 128, KT, 128], bf16, kind="ExternalInput")
    gb = nc.dram_tensor("gb", [128, 2, OT], f32, kind="ExternalInput")
    yt = nc.dram_tensor("yt", [OUTc, Bc], f16, kind="ExternalOutput")

    # Collective bounce buffers per stat group: [mean/4, E[y^2]/4].
    ccin = [
        nc.dram_tensor(f"ccin{q}", [128, 2 * GS[q]], f32) for q in range(NSPLIT)
    ]
    # Local (non-shared) outputs: shared-output collectives need >4-core
    # groups; the payload is tiny so the local-output path is fine.
    ccout = [
        nc.dram_tensor(f"ccout{q}", [n_group * 128, 2 * GS[q]], f32)
        for q in range(NSPLIT)
    ]

    with tile.TileContext(nc) as tc:
        with (
            tc.tile_pool(name="big", bufs=1) as big,
            tc.tile_pool(name="xs", bufs=3) as xsp,
            tc.tile_pool(name="ws", bufs=3) as wsp,
            tc.tile_pool(name="sw", bufs=3) as swp,
            tc.tile_pool(name="ps", bufs=2, space="PSUM") as psp,
            tc.tile_pool(name="st", bufs=2) as stp,
            tc.tile_pool(name="outp", bufs=3) as outp,
        ):
            # Standing tensors
            sxT = big.tile([128, KT, Bc], fp8)          # sign(x)/2, K-major
            yTt = big.tile([128, OT, Bc], f16)          # y.T/2 (exact in fp16)
            mvT = big.tile([128, 2, OT], f32)           # per-core [mean, var]
            gbt = big.tile([128, 2, OT], f32)           # [gamma; beta]
            scal = big.tile([128, OT], f32)             # gamma * rstd
            nbias = big.tile([128, OT], f32)            # beta - mean * scal
            grTs = [None] * NSPLIT                      # gathered stats tiles

            def w_chain(ot):
                # two half-K chunks for finer DMA/ACT pipelining
                swt = swp.tile([128, KT, 128], fp8, tag="swt", name="swt")
                hk = KT // 2
                for h in range(2):
                    wst = wsp.tile([128, hk, 128], bf16, tag="wst", name="wst")
                    nc.sync.dma_start(
                        out=wst[:], in_=w2[ot, :, h * hk : (h + 1) * hk, :]
                    )
                    nc.scalar.sign(swt[:, h * hk : (h + 1) * hk, :], wst[:])
                return swt

            def mm_tile(ot, swt):
                psums = [
                    psp.tile([128, NB], f32, tag=f"ps{bt}", name=f"psum{bt}")
                    for bt in range(BT)
                ]
                # kp-outer: each stationary load is reused across BT b-tiles;
                # also consumes the x k-pairs progressively during startup.
                for kp in range(KP):
                    for bt in range(BT):
                        nc.tensor.matmul(
                            psums[bt][:],
                            lhsT=swt[:, 2 * kp : 2 * kp + 2, :],
                            rhs=sxT[:, 2 * kp : 2 * kp + 2, ts(bt, NB)],
                            start=(kp == 0),
                            stop=(kp == KP - 1),
                            perf_mode=mybir.MatmulPerfMode.DoubleRow,
                        )
                # Drain PSUM on ACT (the DVE queue carries 68us of x-sign
                # work early on; draining there would block PSUM reuse and
                # stall the PE). bn_stats reads the f16 copy from SBUF
                # instead — y/2 is an integer <= 2048, exact in f16.
                for bt in range(BT):
                    nc.scalar.copy(yTt[:, ot, ts(bt, NB)], psums[bt][:])
                st6 = stp.tile([128, BT, 6], f32, tag="st6", name="st6", bufs=4)
                for bt in range(BT):
                    nc.vector.bn_stats(st6[:, bt, :], yTt[:, ot, ts(bt, NB)])
                return nc.vector.bn_aggr(mvT[:, :, ot], st6[:])

            def stats_pre(q):
                """Per-core partial stats -> AllGather, right after group q's
                matmuls."""
                o0, HOT = GO[q], GS[q]
                osl = slice(o0, o0 + HOT)
                arT = stp.tile([128, 2, HOT], f32, tag="arT", name="arT")
                tmp = stp.tile([128, HOT], f32, tag="tmp_ar", name="tmp_ar")
                nc.vector.tensor_scalar_mul(arT[:, 0, :], mvT[:, 0, osl], 1.0 / n_group)
                nc.vector.tensor_mul(tmp[:], mvT[:, 0, osl], mvT[:, 0, osl])
                nc.vector.tensor_add(tmp[:], tmp[:], mvT[:, 1, osl])
                nc.vector.tensor_scalar_mul(arT[:, 1, :], tmp[:], 1.0 / n_group)
                nc.sync.dma_start(out=ccin[q][:], in_=arT[:])
                nc.gpsimd.collective_compute(
                    "AllGather",
                    mybir.AluOpType.bypass,
                    replica_groups=groups,
                    ins=[ccin[q][:]],
                    outs=[ccout[q][:]],
                )
                grA = big.tile([128, n_group, 2, HOT], f32, name=f"grA{q}")
                # SWDGE readback keeps the Sync HWDGE queue free for the
                # next group's weight loads (queues are in-order).
                nc.gpsimd.dma_start(
                    out=grA[:],
                    in_=ccout[q][:].rearrange("(r p) j -> p r j", p=128),
                )
                grTs[q] = grA

            def stats_post(q, anchor=None):
                """Global stats -> scale/bias for group q (anchored a few
                tiles after its collective was issued)."""
                o0, HOT = GO[q], GS[q]
                osl = slice(o0, o0 + HOT)
                grA = grTs[q]
                grT = stp.tile([128, 2, HOT], f32, tag="grT", name="grT")
                first = nc.vector.tensor_reduce(
                    grT[:],
                    grA[:].rearrange("p r two h -> p two h r"),
                    axis=mybir.AxisListType.X,
                    op=mybir.AluOpType.add,
                )
                if anchor is not None:
                    # The scheduler's cost model doesn't know collective
                    # latency; without this ordering edge it hoists the
                    # post-collective math ahead of the running group's PSUM
                    # drains on the in-order DVE queue, stalling the PE.
                    add_dep_helper(first.ins, anchor.ins, sync=False,
                                   reason="post-AR math after current group")
                gmean = grT[:, 0, :]
                gvar = stp.tile([128, HOT], f32, tag="gvar", name="gvar")
                veps = stp.tile([128, HOT], f32, tag="veps", name="veps")
                nc.vector.tensor_mul(gvar[:], gmean, gmean)
                nc.vector.tensor_sub(gvar[:], grT[:, 1, :], gvar[:])
                nc.vector.tensor_scalar_add(veps[:], gvar[:], BN_EPS / 4.0)
                sq = stp.tile([128, HOT], f32, tag="sq", name="sq")
                nc.scalar.sqrt(sq[:], veps[:])
                r = stp.tile([128, HOT], f32, tag="r", name="rstd")
                nc.vector.reciprocal(r[:], sq[:])
                t2 = stp.tile([128, HOT], f32, tag="t2", name="t2")
                for _ in range(2):  # Newton: r <- r * (1.5 - 0.5 * veps * r^2)
                    nc.vector.tensor_mul(t2[:], veps[:], r[:])
                    nc.vector.tensor_mul(t2[:], t2[:], r[:])
                    nc.vector.tensor_scalar(t2[:], t2[:], -0.5, 1.5,
                                            op0=mybir.AluOpType.mult,
                                            op1=mybir.AluOpType.add)
                    nc.vector.tensor_mul(r[:], r[:], t2[:])
                nc.vector.tensor_mul(scal[:, osl], gbt[:, 0, osl], r[:])
                nc.vector.tensor_mul(t2[:], gmean, scal[:, osl])
                nc.vector.tensor_sub(nbias[:, osl], gbt[:, 1, osl], t2[:])

            def norm_group(q):
                # DVE mul-add in fp16 (2x mode). Stores alternate between the
                # ACT and Sync HWDGE queues.
                for ot in range(GO[q], GO[q] + GS[q]):
                    ob = outp.tile([128, Bc], f16, tag="ob", name="ob")
                    nc.vector.tensor_scalar(
                        ob[:],
                        yTt[:, ot, :],
                        scal[:, ot : ot + 1],
                        nbias[:, ot : ot + 1],
                        op0=mybir.AluOpType.mult,
                        op1=mybir.AluOpType.add,
                    )
                    eng = nc.scalar if ot % 2 else nc.sync
                    eng.dma_start(out=yt[ts(ot, 128), :], in_=ob[:])

            # ---- emission order == scheduling priority ----
            # ot=0 weight chain first so the PE can start ASAP
            swt_next = w_chain(0)

            # x sign on DVE: one chunk per k tile, consumed progressively by
            # the kp-outer matmul order of the first output tile. x DMA rides
            # the ACT HWDGE queue so it never queues behind weight loads
            # (which ride Sync); the trigger instructions are cheap and all
            # issue before the first w-sign of tile 1.
            for ks in range(KT):
                xst = xsp.tile([128, Bc], fp8e5, tag="xst", name="xst")
                nc.scalar.dma_start(out=xst[:], in_=xt[ts(ks, 128), :])
                nc.vector.tensor_scalar(
                    sxT[:, ks, :], xst[:], 0.0, 0.5,
                    op0=mybir.AluOpType.is_ge,
                    op1=mybir.AluOpType.subtract,
                )

            nc.sync.dma_start(out=gbt[:], in_=gb[:])

            # post/norm for group q anchored late enough that its collective
            # has certainly completed, early enough to overlap matmuls.
            anchor_ot = {}
            for q in range(1, NSPLIT):
                off = 2 if q == 1 else 1
                a = min(GO[q] + off, OT - 1)
                anchor_ot.setdefault(a, []).append(q - 1)
            anchor_ot.setdefault(OT - 1, []).append(NSPLIT - 1)

            aggrs = []
            for q in range(NSPLIT):
                for ot in range(GO[q], GO[q] + GS[q]):
                    swt = swt_next
                    if ot + 1 < OT:
                        swt_next = w_chain(ot + 1)
                    aggrs.append(mm_tile(ot, swt))
                    if ot == GO[q] + GS[q] - 1:
                        stats_pre(q)
                    for pq in anchor_ot.get(ot, []):
                        stats_post(pq, anchor=aggrs[ot])
                        norm_group(pq)

    nc.finalize()
    return nc


def shard_inputs(x, w, gamma, beta, kb=KB_SHARD, ko=KO_SHARD):
    B, IN = x.shape
    OUT = w.shape[0]
    Bc = B // kb
    OUTc = OUT // ko
    KT, OT = IN // 128, OUTc // 128
    xts = []
    for ib in range(kb):
        xts.append(np.ascontiguousarray(
            x[ib * Bc : (ib + 1) * Bc].T.astype(ml_dtypes.float8_e5m2)
        ))
    wgs = []
    for io in range(ko):
        ws = w[io * OUTc : (io + 1) * OUTc]
        w2 = np.ascontiguousarray(
            ws.reshape(OT, 128, KT, 128).transpose(0, 3, 2, 1)
            .astype(ml_dtypes.bfloat16)
        )
        gbp = np.ascontiguousarray(np.stack(
            [gamma[io * OUTc : (io + 1) * OUTc].reshape(OT, 128).T,
             beta[io * OUTc : (io + 1) * OUTc].reshape(OT, 128).T],
            axis=1,
        )).astype(np.float32)
        wgs.append((w2, gbp))
    in_maps = []
    for c in range(kb * ko):
        io, ib = c // kb, c % kb
        in_maps.append({"xt": xts[ib], "w2": wgs[io][0], "gb": wgs[io][1]})
    return in_maps


_NC_CACHE = {}


def kernel(x, w, gamma, beta):
    x = np.asarray(x)
    w = np.asarray(w)
    gamma = np.asarray(gamma)
    beta = np.asarray(beta)
    B, IN = x.shape
    OUT = w.shape[0]

    key = (B, IN, OUT)
    if key not in _NC_CACHE:
        _NC_CACHE[key] = build(B, IN, OUT)
    nc = _NC_CACHE[key]

    in_maps = shard_inputs(x, w, gamma, beta)
    res = run_bass_kernel_spmd(nc, in_maps, list(range(N_CORES)))
    Bc, OUTc = B // KB_SHARD, OUT // KO_SHARD
    out = np.empty((B, OUT), np.float32)
    for c in range(N_CORES):
        io, ib = c // KB_SHARD, c % KB_SHARD
        out[ib * Bc : (ib + 1) * Bc, io * OUTc : (io + 1) * OUTc] = (
            res.results[c]["yt"].T.astype(np.float32)
        )
    return out


if __name__ == "__main__":
    rng = np.random.default_rng(0)
    B, IN, OUT = 8192, 4096, 4096
    x = rng.standard_normal((B, IN)).astype(np.float32)
    w = rng.standard_normal((OUT, IN)).astype(np.float32)
    gamma = np.ones(OUT, np.float32)
    beta = np.zeros(OUT, np.float32)
    out = kernel(x, w, gamma, beta)
    print(out.shape, out.dtype)


# revision 14
# speedup vs baseline: 1.0252x; 1.0252x over previous
"""Trainium2 Bass kernel for BinaryLinearUnit:
    y = sign(x) @ sign(w).T ; BatchNorm1d(train) ; * gamma + beta

Strategy: 2D sharding over 8 NeuronCores — 4 batch shards x 2
output-feature shards. Each core computes y.T for its [2048 batch x
2048 out-features] block with an FP8 (DoubleRow) matmul. Versus pure
data-parallel this cuts per-core HBM traffic (the dominant baseline
cost) from ~100MB to ~32MB:
  - x ships as fp8e5m2 (sign-preserving for N(0,1) values up to a
    ~6e-6 fraction that rounds to 0; adds ~2.5e-3 rel err, well under
    tolerance), K-major: 8MB/core.
  - w ships as bf16 (bf16 cast preserves sign exactly), K-major
    packed: 16MB/core.
  - y_hat ships back as fp16 (~5e-4 rel err): 8MB/core.

Signs: x-sign on DVE via one tensor_scalar (is_ge 0, sub 0.5) giving
{-0.5,+0.5} fp8 — BatchNorm cancels any constant scale of y exactly,
so +-0.5 works as well as +-1 and needs a single instruction. w-sign
on ACT (scalar.sign) giving +-1 fp8. PSUM accumulation is fp32-exact
(y/2 is a sum of +-0.5 with |y|<=4096, and even y is exact in fp16).

BN batch stats need cross-core reduction only within each group of 4
cores that shares the same output-feature shard: partial [mean, E[y^2]]
per channel are AllGathered over replica groups [[0-3],[4-7]] and
summed locally. The output tiles are processed in NSPLIT stat groups;
each group's collective is issued as soon as its matmuls finish and its
post-collective math + normalization are anchored a few tiles later,
so all BN work except the last group's overlaps the remaining matmuls.

Engine assignment: PE matmuls | ACT w-sign + sqrt + half the output
stores | DVE x-sign, bn_stats, psum->f16 copy, stats math, normalize,
x-input DMA queue | GpSimd collectives + readback | Sync w/gb DMA +
half the output stores.
"""

import numpy as np
import ml_dtypes

import concourse.bass as bass
import concourse.mybir as mybir
import concourse.tile as tile
from concourse import bacc
from concourse.bass import ts
from concourse.bass_utils import run_bass_kernel_spmd
from concourse.tile_rust import add_dep_helper

N_CORES = 8
KB_SHARD = 4            # batch shards
KO_SHARD = 2            # output-feature shards
BN_EPS = 1e-5

f32 = mybir.dt.float32
f16 = mybir.dt.float16
bf16 = mybir.dt.bfloat16
fp8 = mybir.dt.float8e4
fp8e5 = mybir.dt.float8e5


def build(B, IN, OUT, kb=KB_SHARD, ko=KO_SHARD):
    """Per-core SPMD module. Core c handles batch shard c%kb and
    out-feature shard c//kb. Shapes: x [B, IN], w [OUT, IN]."""
    Bc = B // kb            # batch rows per core
    OUTc = OUT // ko        # out features per core
    KT = IN // 128          # k tiles (contraction)
    KP = KT // 2            # fp8 DoubleRow consumes k-pairs
    OT = OUTc // 128        # output-feature tiles per core
    NB = 512                # matmul free dim / psum bank width
    BT = Bc // NB           # b tiles per core
    n_group = N_CORES // ko  # cores sharing one out-feature shard
    groups = [[g * n_group + i for i in range(n_group)] for g in range(ko)]

    # BN stat groups over the output tiles: earlier groups' collectives
    # overlap remaining matmuls. Collectives serialize on the single CC
    # stream and cost ~21us when they absorb inter-core skew, so the
    # second-to-last group ends 2 tiles (~27us) before the matmuls do —
    # hiding its latency — and the last group is small.
    GS = [6, 6, 2, 2] if OT == 16 else [OT - OT // 2, OT // 2]
    NSPLIT = len(GS)
    GO = [sum(GS[:q]) for q in range(NSPLIT)]

    nc = bacc.Bacc("TRN2", target_bir_lowering=False, debug=False,
                   num_devices=N_CORES)

    # Per-core external I/O (host pre-transposed, K-major):
    #   xt[k, b] = x[(c%kb)*Bc + b, k]          as fp8e5m2
    #   w2[ot, p, ks, o] = w[(c//kb)*OUTc + ot*128 + o, ks*128 + p]  bf16
    #   yt[o, b] = out[(c%kb)*Bc + b, (c//kb)*OUTc + o]  fp16
    xt = nc.dram_tensor("xt", [IN, Bc], fp8e5, kind="ExternalInput")
    w2 = nc.dram_tensor("w2", [OT, 128, KT, 128], bf16, kind="ExternalInput")
    gb = nc.dram_tensor("gb", [128, 2, OT], f32, kind="ExternalInput")
    yt = nc.dram_tensor("yt", [OUTc, Bc], f16, kind="ExternalOutput")

    # Collective bounce buffers per stat group: [mean/4, E[y^2]/4].
    ccin = [
        nc.dram_tensor(f"ccin{q}", [128, 2 * GS[q]], f32) for q in range(NSPLIT)
    ]
    # Local (non-shared) outputs: shared-output collectives need >4-core
    # groups; the payload is tiny so the local-output path is fine.
    ccout = [
        nc.dram_tensor(f"ccout{q}", [n_group * 128, 2 * GS[q]], f32)
        for q in range(NSPLIT)
    ]

    with tile.TileContext(nc) as tc:
        with (
            tc.tile_pool(name="big", bufs=1) as big,
            tc.tile_pool(name="xs", bufs=3) as xsp,
            tc.tile_pool(name="ws", bufs=3) as wsp,
            tc.tile_pool(name="sw", bufs=3) as swp,
            tc.tile_pool(name="ps", bufs=2, space="PSUM") as psp,
            tc.tile_pool(name="st", bufs=2) as stp,
            tc.tile_pool(name="outp", bufs=3) as outp,
        ):
            # Standing tensors
            sxT = big.tile([128, KT, Bc], fp8)          # sign(x)/2, K-major
            yTt = big.tile([128, OT, Bc], f16)          # y.T/2 (exact in fp16)
            mvT = big.tile([128, 2, OT], f32)           # per-core [mean, var]
            gbt = big.tile([128, 2, OT], f32)           # [gamma; beta]
            scal = big.tile([128, OT], f32)             # gamma * rstd
            nbias = big.tile([128, OT], f32)            # beta - mean * scal
            grTs = [None] * NSPLIT                      # gathered stats tiles

            def w_chain(ot):
                # two half-K chunks for finer DMA/ACT pipelining
                swt = swp.tile([128, KT, 128], fp8, tag="swt", name="swt")
                hk = KT // 2
                for h in range(2):
                    wst = wsp.tile([128, hk, 128], bf16, tag="wst", name="wst")
                    nc.sync.dma_start(
                        out=wst[:], in_=w2[ot, :, h * hk : (h + 1) * hk, :]
                    )
                    nc.scalar.sign(swt[:, h * hk : (h + 1) * hk, :], wst[:])
                return swt

            def mm_tile(ot, swt):
                psums = [
                    psp.tile([128, NB], f32, tag=f"ps{bt}", name=f"psum{bt}")
                    for bt in range(BT)
                ]
                # kp-outer: each stationary load is reused across BT b-tiles;
                # also consumes the x k-pairs progressively during startup.
                for kp in range(KP):
                    for bt in range(BT):
                        nc.tensor.matmul(
                            psums[bt][:],
                            lhsT=swt[:, 2 * kp : 2 * kp + 2, :],
                            rhs=sxT[:, 2 * kp : 2 * kp + 2, ts(bt, NB)],
                            start=(kp == 0),
                            stop=(kp == KP - 1),
                            perf_mode=mybir.MatmulPerfMode.DoubleRow,
                        )
                # Drain PSUM on DVE. (Tried ACT: every ot then ran at
                # 13.0us instead of the 10.7us the PE sustains when the
                # drain reads ride DVE — see transcript notes.)
                st6 = stp.tile([128, BT, 6], f32, tag="st6", name="st6", bufs=4)
                for bt in range(BT):
                    nc.vector.bn_stats(st6[:, bt, :], psums[bt][:])
                    nc.vector.tensor_copy(yTt[:, ot, ts(bt, NB)], psums[bt][:])
                return nc.vector.bn_aggr(mvT[:, :, ot], st6[:])

            def stats_pre(q):
                """Per-core partial stats -> AllGather, right after group q's
                matmuls."""
                o0, HOT = GO[q], GS[q]
                osl = slice(o0, o0 + HOT)
                arT = stp.tile([128, 2, HOT], f32, tag="arT", name="arT")
                tmp = stp.tile([128, HOT], f32, tag="tmp_ar", name="tmp_ar")
                nc.vector.tensor_scalar_mul(arT[:, 0, :], mvT[:, 0, osl], 1.0 / n_group)
                nc.vector.tensor_mul(tmp[:], mvT[:, 0, osl], mvT[:, 0, osl])
                nc.vector.tensor_add(tmp[:], tmp[:], mvT[:, 1, osl])
                nc.vector.tensor_scalar_mul(arT[:, 1, :], tmp[:], 1.0 / n_group)
                nc.sync.dma_start(out=ccin[q][:], in_=arT[:])
                nc.gpsimd.collective_compute(
                    "AllGather",
                    mybir.AluOpType.bypass,
                    replica_groups=groups,
                    ins=[ccin[q][:]],
                    outs=[ccout[q][:]],
                )
                grA = big.tile([128, n_group, 2, HOT], f32, name=f"grA{q}")
                # SWDGE readback keeps the Sync HWDGE queue free for the
                # next group's weight loads (queues are in-order) — except
                # the last group, where Sync is idle and HWDGE is ~3us
                # faster than the SWDGE path, straight into the tail.
                eng = nc.sync if q == NSPLIT - 1 else nc.gpsimd
                eng.dma_start(
                    out=grA[:],
                    in_=ccout[q][:].rearrange("(r p) j -> p r j", p=128),
                )
                grTs[q] = grA

            def stats_post(q, anchor=None):
                """Global stats -> scale/bias for group q (anchored a few
                tiles after its collective was issued)."""
                o0, HOT = GO[q], GS[q]
                osl = slice(o0, o0 + HOT)
                grA = grTs[q]
                grT = stp.tile([128, 2, HOT], f32, tag="grT", name="grT")
                first = nc.vector.tensor_reduce(
                    grT[:],
                    grA[:].rearrange("p r two h -> p two h r"),
                    axis=mybir.AxisListType.X,
                    op=mybir.AluOpType.add,
                )
                if anchor is not None:
                    # The scheduler's cost model doesn't know collective
                    # latency; without this ordering edge it hoists the
                    # post-collective math ahead of the running group's PSUM
                    # drains on the in-order DVE queue, stalling the PE.
                    add_dep_helper(first.ins, anchor.ins, sync=False,
                                   reason="post-AR math after current group")
                gmean = grT[:, 0, :]
                gvar = stp.tile([128, HOT], f32, tag="gvar", name="gvar")
                veps = stp.tile([128, HOT], f32, tag="veps", name="veps")
                nc.vector.tensor_mul(gvar[:], gmean, gmean)
                nc.vector.tensor_sub(gvar[:], grT[:, 1, :], gvar[:])
                nc.vector.tensor_scalar_add(veps[:], gvar[:], BN_EPS / 4.0)
                # sqrt + plain reciprocal, no Newton refine: the approx
                # error is far inside the 2e-2 gate and the serial Newton
                # chain cost ~2us of tail latency per group.
                sq = stp.tile([128, HOT], f32, tag="sq", name="sq")
                nc.scalar.sqrt(sq[:], veps[:])
                r = stp.tile([128, HOT], f32, tag="r", name="rstd")
                nc.vector.reciprocal(r[:], sq[:])
                t2 = stp.tile([128, HOT], f32, tag="t2", name="t2")
                nc.vector.tensor_mul(scal[:, osl], gbt[:, 0, osl], r[:])
                nc.vector.tensor_mul(t2[:], gmean, scal[:, osl])
                nc.vector.tensor_sub(nbias[:, osl], gbt[:, 1, osl], t2[:])

            def norm_group(q):
                # Normalize on GPSIMD (idle between collectives) so the DVE
                # stays clear for PSUM drains — concurrent DVE norm work was
                # correlated with the PE slowing from 10.7 to 13.0us/tile.
                # Stores alternate between the ACT and Sync HWDGE queues.
                for ot in range(GO[q], GO[q] + GS[q]):
                    ob = outp.tile([128, Bc], f16, tag="ob", name="ob")
                    nc.gpsimd.tensor_scalar(
                        ob[:],
                        yTt[:, ot, :],
                        scal[:, ot : ot + 1],
                        nbias[:, ot : ot + 1],
                        op0=mybir.AluOpType.mult,
                        op1=mybir.AluOpType.add,
                    )
                    eng = nc.scalar if ot % 2 else nc.sync
                    eng.dma_start(out=yt[ts(ot, 128), :], in_=ob[:])

            # ---- emission order == scheduling priority ----
            # ot=0 weight chain first so the PE can start ASAP
            swt_next = w_chain(0)

            # x sign on DVE, one chunk per k-PAIR (the DoubleRow consumption
            # unit), consumed progressively by the kp-outer matmul order of
            # the first output tile. x DMA rides the ACT HWDGE queue so it
            # never queues behind weight loads (which ride Sync). Chunks are
            # 2 k-tiles per trigger: a HWDGE trigger costs ~0.6us of engine
            # time, so 32 single-tile triggers would throttle the x stream.
            for kp in range(KP):
                xst = xsp.tile([128, 2, Bc], fp8e5, tag="xst", name="xst")
                nc.scalar.dma_start(
                    out=xst[:],
                    in_=xt[ts(kp, 256), :].rearrange("(f p) b -> p f b", p=128),
                )
                nc.vector.tensor_scalar(
                    sxT[:, 2 * kp : 2 * kp + 2, :], xst[:], 0.0, 0.5,
                    op0=mybir.AluOpType.is_ge,
                    op1=mybir.AluOpType.subtract,
                )

            nc.sync.dma_start(out=gbt[:], in_=gb[:])

            # post/norm for group q anchored late enough that its collective
            # has certainly completed, early enough to overlap matmuls.
            anchor_ot = {}
            for q in range(1, NSPLIT):
                off = 2 if q == 1 else 1
                a = min(GO[q] + off, OT - 1)
                anchor_ot.setdefault(a, []).append(q - 1)
            anchor_ot.setdefault(OT - 1, []).append(NSPLIT - 1)

            aggrs = []
            for q in range(NSPLIT):
                for ot in range(GO[q], GO[q] + GS[q]):
                    swt = swt_next
                    if ot + 1 < OT:
                        swt_next = w_chain(ot + 1)
                    aggrs.append(mm_tile(ot, swt))
                    if ot == GO[q] + GS[q] - 1:
                        stats_pre(q)
                    for pq in anchor_ot.get(ot, []):
                        stats_post(pq, anchor=aggrs[ot])
                        norm_group(pq)

    nc.finalize()
    return nc


def shard_inputs(x, w, gamma, beta, kb=KB_SHARD, ko=KO_SHARD):
    B, IN = x.shape
    OUT = w.shape[0]
    Bc = B // kb
    OUTc = OUT // ko
    KT, OT = IN // 128, OUTc // 128
    xts = []
    for ib in range(kb):
        xts.append(np.ascontiguousarray(
            x[ib * Bc : (ib + 1) * Bc].T.astype(ml_dtypes.float8_e5m2)
        ))
    wgs = []
    for io in range(ko):
        ws = w[io * OUTc : (io + 1) * OUTc]
        w2 = np.ascontiguousarray(
            ws.reshape(OT, 128, KT, 128).transpose(0, 3, 2, 1)
            .astype(ml_dtypes.bfloat16)
        )
        gbp = np.ascontiguousarray(np.stack(
            [gamma[io * OUTc : (io + 1) * OUTc].reshape(OT, 128).T,
             beta[io * OUTc : (io + 1) * OUTc].reshape(OT, 128).T],
            axis=1,
        )).astype(np.float32)
        wgs.append((w2, gbp))
    in_maps = []
    for c in range(kb * ko):
        io, ib = c // kb, c % kb
        in_maps.append({"xt": xts[ib], "w2": wgs[io][0], "gb": wgs[io][1]})
    return in_maps


_NC_CACHE = {}


def kernel(x, w, gamma, beta):
    x = np.asarray(x)
    w = np.asarray(w)
    gamma = np.asarray(gamma)
    beta = np.asarray(beta)
    B, IN = x.shape
    OUT = w.shape[0]

    key = (B, IN, OUT)
    if key not in _NC_CACHE:
        _NC_CACHE[key] = build(B, IN, OUT)
    nc = _NC_CACHE[key]

    in_maps = shard_inputs(x, w, gamma, beta)
    res = run_bass_kernel_spmd(nc, in_maps, list(range(N_CORES)))
    Bc, OUTc = B // KB_SHARD, OUT // KO_SHARD
    out = np.empty((B, OUT), np.float32)
    for c in range(N_CORES):
        io, ib = c // KB_SHARD, c % KB_SHARD
        out[ib * Bc : (ib + 1) * Bc, io * OUTc : (io + 1) * OUTc] = (
            res.results[c]["yt"].T.astype(np.float32)
        )
    return out


if __name__ == "__main__":
    rng = np.random.default_rng(0)
    B, IN, OUT = 8192, 4096, 4096
    x = rng.standard_normal((B, IN)).astype(np.float32)
    w = rng.standard_normal((OUT, IN)).astype(np.float32)
    gamma = np.ones(OUT, np.float32)
    beta = np.zeros(OUT, np.float32)
    out = kernel(x, w, gamma, beta)
    print(out.shape, out.dtype)
